# revision 24
# baseline (speedup 1.0000x reference)
"""AttentionPooling Trainium2 Bass kernel.

Full inputs in, full outputs out. Data-parallel over batch across 8 cores
(2 batches per core). Algebraic restructuring done on host (all O(D^2)):

  scores[b,h,s] = scale * (q_h @ Wk_h) . tokens[b,s]  -> single [D,H] matrix qkT
  pooled[b]     = blockdiag-by-head Wv @ (attn^T tokens) -> V-projection
                  deferred until after the sequence reduction.

Device work per batch: stream token chunks once (memory roofline),
PE-transpose [128,128] tiles for the d-contraction scores matmul (bf16 on
the PSUM->SBUF copy), exp (softmax without max subtraction -- exact:
softmax(x) == exp(x)/sum(exp(x)); scores here are O(0.1); the key-padding
mask is folded into the scores as a -1e30 bias via one extra K=1 matmul,
only when the mask is not all-ones), then accumulate attn^T @ tokens in
PSUM across all chunks; the softmax normalizer rides the exp instruction's
accum_out. Tiny projections + LayerNorm run on device at the end.
"""

import numpy as np

B, S, D, H = 16, 4096, 1536, 8
HD = D // H                     # 192
N_CORES = 8
B_LOC = B // N_CORES            # 2 batches per core
NJ = D // 128                   # 12 d-tiles
CT = 128                        # tokens per streamed chunk
EPS = 1e-6

_CACHE = {}


def _build_nc(reps=1, ct=CT, chunk_bufs=3, tt_bufs=2, weights_late=True,
              ablate="none", masked=False, ring_alt=False, copies="mix",
              small_bufs=3):
    import concourse.bacc as bacc
    import concourse.tile as tile
    from concourse import mybir
    from concourse.masks import make_identity

    f32 = mybir.dt.float32
    bf16 = mybir.dt.bfloat16
    Exp = mybir.ActivationFunctionType.Exp
    Sqrt = mybir.ActivationFunctionType.Sqrt

    nsub = ct // 128            # 128-token subtiles per chunk
    nchunk = S // ct            # chunks per batch
    ngrp = (NJ * nsub) // 4     # transpose groups (4 per PSUM bank)

    nc = bacc.Bacc("TRN2", target_bir_lowering=False, debug=False)

    tok = nc.declare_dram_parameter("tok", [B_LOC, S, D], f32, isOutput=False)
    qkt = nc.declare_dram_parameter("qkt", [128, NJ, H], bf16, isOutput=False)
    sbias = nc.declare_dram_parameter("sbias", [H, 1], f32, isOutput=False)
    maskS = nc.declare_dram_parameter("maskS", [1, B_LOC * nchunk, ct], bf16,
                                      isOutput=False)
    wvt = nc.declare_dram_parameter("wvt", [NJ, 128, D], f32, isOutput=False)
    wot = nc.declare_dram_parameter("wot", [NJ, 128, D], f32, isOutput=False)
    bvec = nc.declare_dram_parameter("bvec", [B_LOC, 4, D], f32, isOutput=False)
    out = nc.declare_dram_parameter("out", [B_LOC, D], f32, isOutput=True)

    with tile.TileContext(nc) as tc:
        with tc.tile_pool(name="singles", bufs=1) as singles:
            ident = singles.tile([128, 128], f32)
            make_identity(nc, ident)
            mbneg = singles.tile([128, 1], bf16)
            nc.vector.memset(mbneg, -1e30)
            eps_sb = singles.tile([B_LOC, 1], f32)
            nc.vector.memset(eps_sb, EPS)

            qkt_sb = singles.tile([128, NJ, H], bf16)
            nc.sync.dma_start(out=qkt_sb, in_=qkt.ap())
            sb_sb = singles.tile([H, 1], f32)
            nc.sync.dma_start(out=sb_sb, in_=sbias.ap())
            if masked:
                maskS_sb = singles.tile([1, B_LOC * nchunk, ct], bf16)
                nc.sync.dma_start(out=maskS_sb, in_=maskS.ap())

            # weights: resident for the whole kernel, loaded on the scalar
            # HWDGE ring so they do not block the token stream on sync's ring
            wvt_sb = singles.tile([128, NJ, D], f32)
            wot_sb = singles.tile([128, NJ, D], f32)

            def load_weights():
                for j in range(NJ):
                    nc.scalar.dma_start(out=wvt_sb[:, j, :], in_=wvt.ap()[j])
                for j in range(NJ):
                    nc.scalar.dma_start(out=wot_sb[:, j, :], in_=wot.ap()[j])

            if not weights_late:
                load_weights()

            # lhsT for the V-projection: tpT[d, j, h, b]
            tpT_sb = singles.tile([128, NJ, H, B_LOC], f32)

            def emit_ptrans(carry, smalls, ps_trp):
                """Transpose masked exp(scores) of a finished chunk to
                [s, H] per subtile. Returns the pooling lhsT tile."""
                ps_exp, c, tokc = carry
                ppt = ps_trp.tile([128, nsub * H], f32, tag="trp")
                for sub in range(nsub):
                    nc.tensor.transpose(
                        ppt[:, sub * H:(sub + 1) * H],
                        ps_exp[:, sub * 128:(sub + 1) * 128], ident[:H, :H])
                pt_sb = smalls.tile([128, nsub, H], f32, tag="pt")
                nc.vector.tensor_copy(
                    out=pt_sb.rearrange("p a q -> p (a q)"), in_=ppt)
                return pt_sb

            def emit_poolmm(carry, pt_sb, psum_tpool):
                """Accumulate attn^T @ tokens for a chunk."""
                ps_exp, c, tokc = carry
                for sub in range(nsub):
                    for k in range(3):
                        nc.tensor.matmul(
                            psum_tpool[:, k * 512:(k + 1) * 512],
                            pt_sb[:, sub, :],
                            tokc[:, sub, k * 512:(k + 1) * 512],
                            start=(c == 0 and sub == 0),
                            stop=(c == nchunk - 1 and sub == nsub - 1),
                        )

            for rep in range(reps):
                with (
                    tc.tile_pool(name="chunks", bufs=chunk_bufs) as chunks,
                    tc.tile_pool(name="tts", bufs=tt_bufs) as tts,
                    tc.tile_pool(name="smalls", bufs=small_bufs) as smalls,
                    tc.tile_pool(name="tpp", bufs=1) as tpp,
                    tc.tile_pool(name="ps_big", bufs=1, space="PSUM") as ps_big,
                    tc.tile_pool(name="ps_sc", bufs=2, space="PSUM") as ps_sc,
                    tc.tile_pool(name="ps_tr", bufs=2, space="PSUM") as ps_tr,
                    tc.tile_pool(name="ps_trp", bufs=1, space="PSUM") as ps_trp,
                ):
                    for b in range(B_LOC):
                        if ablate == "none":
                            psum_tpool = ps_big.tile([H, D], f32, tag="tpool")
                            l_parts = smalls.tile([H, nchunk], f32,
                                                  tag="lparts", name="l_parts")
                        else:
                            psum_tpool = l_parts = None

                        # software-pipelined chunk loop: pooling of chunk c-1
                        # is emitted after chunk c's score matmuls so PE never
                        # waits on the exp of the chunk it just scored.
                        carry = None
                        for c in range(nchunk):
                            tokc = chunks.tile([128, nsub, D], f32, tag="tok")
                            src_ap = tok.ap()[b].rearrange(
                                "(g r p) d -> g p r d", r=nsub, p=128)
                            eng = (nc.scalar if (ring_alt and c % 2)
                                   else nc.sync)
                            eng.dma_start(out=tokc, in_=src_ap[c])
                            if ablate == "dma":
                                carry = None
                                continue
                            tt = tts.tile([128, NJ, ct], bf16, tag="tt")
                            tt_flat = tt.rearrange("p j s -> p (j s)")
                            for grp in range(ngrp):
                                ptr = ps_tr.tile([128, 512], f32, tag="tr")
                                for q in range(4):
                                    t = grp * 4 + q
                                    j, sub = divmod(t, nsub)
                                    nc.tensor.transpose(
                                        ptr[:, q * 128:(q + 1) * 128],
                                        tokc[:, sub, j * 128:(j + 1) * 128],
                                        ident,
                                    )
                                dst = tt_flat[:, grp * 512:(grp + 1) * 512]
                                if copies == "dve" or grp % 3 < 2:
                                    nc.vector.tensor_copy(out=dst, in_=ptr)
                                else:
                                    nc.scalar.copy(out=dst, in_=ptr)

                            if carry is not None:
                                pt_c = emit_ptrans(carry, smalls, ps_trp)

                            if ablate == "tr":
                                carry = None
                                continue
                            psc = ps_sc.tile([H, ct], f32, tag="sc")
                            for j in range(NJ):
                                nc.tensor.matmul(
                                    psc, qkt_sb[:, j, :], tt[:, j, :],
                                    start=(j == 0),
                                    stop=(not masked and j == NJ - 1),
                                )
                            if masked:
                                nc.tensor.matmul(
                                    psc,
                                    mbneg[0:1, 0:1].broadcast_to((1, H)),
                                    maskS_sb[0:1, b * nchunk + c, :],
                                    start=False, stop=True,
                                )
                            ps_exp = smalls.tile([H, ct], f32, tag="pexp")
                            nc.scalar.activation(ps_exp, psc, Exp,
                                                 bias=sb_sb, scale=1.0,
                                                 accum_out=l_parts[:, c:c + 1])
                            if carry is not None:
                                emit_poolmm(carry, pt_c, psum_tpool)
                            if ablate == "sc":
                                carry = None
                                continue
                            carry = (ps_exp, c, tokc)
                        if carry is not None:
                            pt_c = emit_ptrans(carry, smalls, ps_trp)
                            emit_poolmm(carry, pt_c, psum_tpool)
                        carry = None

                        if ablate != "none":
                            continue
                        # batch epilogue: normalize and transpose t_pool
                        lsum = smalls.tile([H, 1], f32, tag="lsum")
                        nc.vector.reduce_sum(out=lsum, in_=l_parts,
                                             axis=mybir.AxisListType.X)
                        linv = smalls.tile([H, 1], f32, tag="linv")
                        nc.vector.reciprocal(linv, lsum)
                        tp_sb = tpp.tile([H, D], f32, tag="tp")
                        nc.vector.tensor_scalar_mul(tp_sb, psum_tpool, linv)
                        for j in range(NJ):
                            ptp = ps_trp.tile([128, nsub * H], f32, tag="trp")
                            nc.tensor.transpose(
                                ptp[:, :H], tp_sb[:, j * 128:(j + 1) * 128],
                                ident[:H, :H],
                            )
                            nc.vector.tensor_copy(
                                out=tpT_sb[:, j, :, b], in_=ptp[:, :H]
                            )

                if weights_late and rep == 0:
                    load_weights()

                if ablate != "none":
                    continue

                # ---- core epilogue: projections + layernorm ----
                with (
                    tc.tile_pool(name="epil", bufs=1) as epil,
                    tc.tile_pool(name="ps_epi", bufs=1, space="PSUM") as ps_epi,
                ):
                    bvec_sb = epil.tile([B_LOC, 4, D], f32, tag="bvec")
                    nc.sync.dma_start(out=bvec_sb, in_=bvec.ap())
                    bv2_sb = bvec_sb[:, 0, :]
                    bo2_sb = bvec_sb[:, 1, :]
                    g2_sb = bvec_sb[:, 2, :]
                    be2_sb = bvec_sb[:, 3, :]

                    # V-projection per head, output lands directly in [b, e]
                    # layout (256-f32 stride keeps each matmul in one bank)
                    psum_vp = ps_epi.tile([B_LOC, H, 256], f32, tag="vp")
                    for h in range(H):
                        for j in range(NJ):
                            nc.tensor.matmul(
                                psum_vp[:, h, 0:HD],
                                tpT_sb[:, j, h, :],
                                wvt_sb[:, j, h * HD:(h + 1) * HD],
                                start=(j == 0), stop=(j == NJ - 1),
                            )
                    pooled_sb = epil.tile([B_LOC, H, HD], f32, tag="pooled")
                    nc.vector.tensor_add(
                        pooled_sb, psum_vp[:, :, 0:HD],
                        bv2_sb.rearrange("p (h e) -> p h e", h=H),
                    )
                    pooled_flat = pooled_sb.rearrange("p h e -> p (h e)")

                    # O-projection: transpose pooled, psum_op = pooledT.T @ woT
                    poT_sb = epil.tile([128, NJ, B_LOC], f32, tag="poT")
                    for j in range(NJ):
                        ppo = ps_epi.tile([128, B_LOC], f32, tag="po")
                        nc.tensor.transpose(
                            ppo, pooled_flat[:, j * 128:(j + 1) * 128],
                            ident[:B_LOC, :B_LOC],
                        )
                        nc.vector.tensor_copy(out=poT_sb[:, j, :], in_=ppo)
                    psum_op = ps_epi.tile([B_LOC, D], f32, tag="op")
                    for j in range(NJ):
                        for k in range(3):
                            nc.tensor.matmul(
                                psum_op[:, k * 512:(k + 1) * 512],
                                poT_sb[:, j, :],
                                wot_sb[:, j, k * 512:(k + 1) * 512],
                                start=(j == 0), stop=(j == NJ - 1),
                            )
                    x_sb = epil.tile([B_LOC, D], f32, tag="x")
                    nc.vector.tensor_add(x_sb, psum_op, bo2_sb)

                    # LayerNorm
                    x3 = x_sb.rearrange("p (g q) -> p g q", g=3)
                    stats = epil.tile([B_LOC, 3, 6], f32, tag="stats")
                    for g in range(3):
                        nc.vector.bn_stats(out=stats[:, g, :], in_=x3[:, g, :])
                    mv = epil.tile([B_LOC, 2], f32, tag="mv")
                    nc.vector.bn_aggr(out=mv, in_=stats)
                    sd = epil.tile([B_LOC, 1], f32, tag="sd")
                    nc.scalar.activation(sd, mv[:, 1:2], Sqrt,
                                         bias=eps_sb, scale=1.0)
                    rstd = epil.tile([B_LOC, 1], f32, tag="rstd")
                    nc.vector.reciprocal(rstd, sd)
                    xc = epil.tile([B_LOC, D], f32, tag="xc")
                    nc.vector.tensor_scalar_sub(xc, x_sb, mv[:, 0:1])
                    nc.vector.tensor_scalar_mul(xc, xc, rstd)
                    nc.vector.tensor_mul(xc, xc, g2_sb)
                    nc.vector.tensor_add(xc, xc, be2_sb)
                    nc.sync.dma_start(out=out.ap(), in_=xc)

    nc.compile()
    return nc


def _host_prep(tokens, mask, query, wq, wk, wv, bq, bk, bv, wo, bo, gamma,
               beta, ct=CT):
    """Fold the tiny projections; all O(D^2) work in float64 for accuracy."""
    import ml_dtypes
    scale = 1.0 / np.sqrt(HD)
    q = (np.asarray(query, np.float64) @ np.asarray(wq, np.float64).T
         + np.asarray(bq, np.float64)).reshape(H, HD)
    qk = np.empty((H, D), np.float64)
    sb = np.empty((H, 1), np.float64)
    wk64 = np.asarray(wk, np.float64)
    bk64 = np.asarray(bk, np.float64)
    for h in range(H):
        qk[h] = scale * (q[h] @ wk64[h * HD:(h + 1) * HD, :])
        sb[h, 0] = scale * (q[h] @ bk64[h * HD:(h + 1) * HD])
    # qkt[p, j, h] = qk[h, 128j + p]
    qkt = np.ascontiguousarray(
        qk.T.reshape(NJ, 128, H).transpose(1, 0, 2)).astype(ml_dtypes.bfloat16)

    wvt = np.ascontiguousarray(
        np.asarray(wv, np.float32).T.reshape(NJ, 128, D))
    wot = np.ascontiguousarray(
        np.asarray(wo, np.float32).T.reshape(NJ, 128, D))

    maskf = np.asarray(mask).astype(np.float32)          # [B, S]
    nchunk = S // ct
    # maskS[core][0, b*nchunk + c, i] = 1 - mask[core*B_LOC + b, c*ct + i]
    maskS_all = np.ascontiguousarray(
        (1.0 - maskf).reshape(N_CORES, 1, B_LOC * nchunk, ct)
    ).astype(ml_dtypes.bfloat16)

    bvec = np.ascontiguousarray(np.broadcast_to(
        np.stack([np.asarray(v, np.float32) for v in (bv, bo, gamma, beta)]),
        (B_LOC, 4, D)))

    common = {
        "qkt": qkt,
        "sbias": sb.astype(np.float32),
        "wvt": wvt,
        "wot": wot,
        "bvec": bvec,
    }
    tokens = np.asarray(tokens)
    if tokens.dtype != np.float32:
        tokens = tokens.astype(np.float32)
    in_maps = []
    for core in range(N_CORES):
        m = dict(common)
        m["tok"] = np.ascontiguousarray(
            tokens[core * B_LOC:(core + 1) * B_LOC])
        m["maskS"] = np.ascontiguousarray(maskS_all[core])
        in_maps.append(m)
    return in_maps


def kernel(tokens, mask, query, wq, wk, wv, bq, bk, bv, wo, bo, gamma, beta):
    from concourse.bass_utils import run_bass_kernel_spmd

    masked = not bool(np.all(np.asarray(mask)))
    key = ("nc", masked)
    if key not in _CACHE:
        _CACHE[key] = _build_nc(masked=masked)
    nc = _CACHE[key]
    in_maps = _host_prep(tokens, mask, query, wq, wk, wv, bq, bk, bv,
                         wo, bo, gamma, beta)
    res = run_bass_kernel_spmd(nc, in_maps, list(range(N_CORES)))
    return np.concatenate([res.results[c]["out"] for c in range(N_CORES)],
                          axis=0).astype(np.float32)
